# revision 35
# baseline (speedup 1.0000x reference)
"""GNN mean-aggregation message passing on 8 Trainium2 NeuronCores.

out[t] = mean_{e: tgt[e]==t} x[src[e]]   (0 if no incoming edges)

Strategy (target-sharded, uniform SPMD schedule):
  - Each core owns a contiguous range of 12544 targets (98 groups of 128).
  - Host packs x as bf16 hi|lo pairs -> [N_pad, 128] bf16 (256B rows). The
    hi+lo split recovers ~fp32 precision after the f32 PSUM accumulation.
  - Edges are routed to the owning core, ordered by (supergroup, src-chunk,
    target) and packed into 128-edge slots. Every slot is bound (at compile
    time, uniformly across cores) to a target group g; its edges may only
    reference groups {g, g+1} (spill-chained packing keeps padding ~3%).
  - Sources are gathered straight from HBM with dma_gather (int16 indices,
    4 chunks of 25088 rows to fit the int16 range). Gather calls are striped
    across 4 SWDGE queues so descriptor generation runs on all 8 Q7 cores;
    each call owns its own msgs tile so gathers stream ahead of compute.
  - One matmul per slot: psum[elem, tgt] += msgs^T @ sel, with sel a
    [128, 256] one-hot. sel is built on DVE (tensor_tensor is_equal against
    a materialized iota replica) with a fraction offloaded to the idle
    Scalar engine as relu(1 - |iota - trow|), exact on integers. PSUM holds
    one supergroup of 15 groups as [128,256] half-bank regions; group h's
    spill lands in region h's upper half.
  - Scalar engine zeroes PSUM regions and drains them to SBUF at finalize;
    DVE does the hi+lo+spill adds. The mean divide (and zero-degree mask)
    runs on the host.
"""
import sys

sys.path.insert(0, "/opt/trn_rl_repo")

import numpy as np
import ml_dtypes

bf16 = ml_dtypes.bfloat16

# ---- problem constants (hardcoded per harness contract) ----
N, F, E = 100000, 64, 1600000
P = 128
NCORES = 8
TPC = 12544                  # targets per core (= 98 * 128)
GPC = TPC // P               # 98 groups per core
NCHUNKS = 4
CHUNK = 25088                # source rows per chunk (< 32768 for int16 idx)
NPAD = NCHUNKS * CHUNK       # 100352 padded source rows
GSG = 15                     # groups per supergroup; 15*256 f32 <= 8 PSUM banks
SG_SIZES = [GSG] * (GPC // GSG) + ([GPC % GSG] if GPC % GSG else [])  # [15]*6+[8]
ELEM = 2 * F                 # 128 bf16 per packed row = 256B
SENT = -2.0                  # sentinel row id for pad edges (never matches iota)
SEL_PIECE = 16               # slots per sel tile / DVE is_equal batch
CALL_SLOTS = 32              # slots per dma_gather call (own msgs tile each)
NQ = 4                       # SWDGE queues
SCALAR_SEL = 2               # of every SEL_MOD sel pieces, this many -> Scalar
SEL_MOD = 7

SEL_DVE_INSTS = []
SEL_SCALAR_INSTS = []
MM_ALL = []


def _chunk_order(sched, s):
    return sorted(range(NCHUNKS), key=lambda c: (-len(sched[s][c]), c))


def _host_prep(x, edge_idx):
    """Build per-core device arrays and the shared slot schedule."""
    x = np.asarray(x, np.float32)
    src = np.asarray(edge_idx[0], np.int64)
    tgt = np.asarray(edge_idx[1], np.int64)

    # packed hi|lo bf16 table
    hi = x.astype(bf16)
    lo = (x - hi.astype(np.float32)).astype(bf16)
    xp = np.zeros((NPAD, ELEM), bf16)
    xp[:N, :F] = hi
    xp[:N, F:] = lo

    cnt = np.bincount(tgt, minlength=NCORES * TPC).astype(np.float32)
    inv_cnt = 1.0 / np.maximum(cnt, 1.0)
    inv_cnt[cnt == 0.0] = 0.0          # zero-degree targets output 0

    core = tgt // TPC
    chunk = src // CHUNK
    gl = (tgt // P) - core * GPC          # local group 0..97
    sg = np.minimum(gl // GSG, len(SG_SIZES) - 1)
    order = np.lexsort((tgt, chunk, sg, core))

    nsg = len(SG_SIZES)
    bin_id = (core * nsg + sg) * NCHUNKS + chunk
    bin_sizes = np.bincount(bin_id, minlength=NCORES * nsg * NCHUNKS)
    bin_starts = np.zeros(NCORES * nsg * NCHUNKS + 1, np.int64)
    np.cumsum(bin_sizes, out=bin_starts[1:])

    gl_sorted = gl[order]
    tgt_sorted = tgt[order]
    src_sorted = src[order]
    chunk_sorted = chunk[order]

    # ---- build shared schedule: per (sg, c) the block label list ----
    sched = []          # sched[sg][c] = np.array of block labels h (bin-local)
    for s in range(nsg):
        gs = SG_SIZES[s]
        row = []
        for c in range(NCHUNKS):
            e_kh = np.zeros((NCORES, gs), np.int64)
            for k in range(NCORES):
                b = (k * nsg + s) * NCHUNKS + c
                seg = gl_sorted[bin_starts[b]:bin_starts[b + 1]] - s * GSG
                if seg.size:
                    e_kh[k] = np.bincount(seg, minlength=gs)
            labels = []
            r = e_kh[:, 0].astype(np.int64)
            for h in range(gs):
                s_h = int(np.ceil(r / P).max())
                labels.extend([h] * s_h)
                cap = s_h * P - r
                if h + 1 < gs:
                    r = np.maximum(0, e_kh[:, h + 1] - cap)
                else:
                    assert (cap >= 0).all()
            row.append(np.asarray(labels, np.int64))
        sched.append(row)

    tot_slots = sum(len(row[c]) for row in sched for c in range(NCHUNKS))
    tot = tot_slots * P

    # ---- per-core edge placement into the uniform slot stream ----
    src_local = np.zeros((NCORES, tot), np.int16)
    trow = np.full((NCORES, tot), SENT, np.float32)
    for k in range(NCORES):
        base = 0
        for s in range(nsg):
            for c in _chunk_order(sched, s):
                labels = sched[s][c]
                b = (k * nsg + s) * NCHUNKS + c
                lo_i, hi_i = bin_starts[b], bin_starts[b + 1]
                garr = gl_sorted[lo_i:hi_i] - s * GSG
                p = 0
                for bi, h in enumerate(labels):
                    upper = np.searchsorted(garr, h + 1, side="right")
                    take = min(P, upper - p)
                    if take > 0:
                        sl = slice(lo_i + p, lo_i + p + take)
                        pos = base + bi * P
                        src_local[k, pos:pos + take] = (
                            src_sorted[sl] - chunk_sorted[sl] * CHUNK
                        ).astype(np.int16)
                        trow[k, pos:pos + take] = (
                            tgt_sorted[sl] % P + P * (garr[p:p + take] - h)
                        ).astype(np.float32)
                        p += take
                assert p == hi_i - lo_i, (
                    f"core {k} sg {s} c {c}: placed {p} of {hi_i - lo_i}"
                )
                base += len(labels) * P
        assert base == tot

    # device layouts
    idx_dev = [
        np.tile(src_local[k].reshape(tot // 16, 16).T, (8, 1)).copy()
        for k in range(NCORES)
    ]
    trow_dev = [
        arr.reshape(tot // P, P).T.astype(bf16).copy()   # [P, tot//P] bf16
        for arr in trow
    ]
    return xp, idx_dev, trow_dev, inv_cnt, sched, tot


def _build_program(sched, tot):
    from concourse import bacc, mybir, tile

    nsg = len(SG_SIZES)
    SEL_DVE_INSTS.clear()
    SEL_SCALAR_INSTS.clear()
    MM_ALL.clear()

    nc = bacc.Bacc(None, target_bir_lowering=False, debug=False,
                   num_swdge_queues=NQ)
    t_x = nc.dram_tensor("xp", [NPAD, ELEM], mybir.dt.bfloat16, kind="ExternalInput")
    t_idx = nc.dram_tensor("idx", [P, tot // 16], mybir.dt.int16, kind="ExternalInput")
    t_trow = nc.dram_tensor("trow", [P, tot // P], mybir.dt.bfloat16, kind="ExternalInput")
    t_out = nc.dram_tensor("out", [F, TPC], mybir.dt.float32, kind="ExternalOutput")

    with tile.TileContext(nc) as tc:
        with (
            tc.tile_pool(name="const", bufs=1) as cpool,
            tc.tile_pool(name="msgs", bufs=8) as mpool,
            tc.tile_pool(name="sel", bufs=7) as spool,
            tc.tile_pool(name="sq", bufs=4) as qpool,
            tc.tile_pool(name="stage", bufs=2) as stpool,
            tc.tile_pool(name="fold", bufs=4) as fpool,
            tc.tile_pool(name="psum", bufs=1, space="PSUM") as ppool,
        ):
            idx_t = cpool.tile([P, tot // 16], mybir.dt.int16)
            trow_t = cpool.tile([P, tot // P], mybir.dt.bfloat16)
            ntrow_t = cpool.tile([P, tot // P], mybir.dt.float32)
            iota_i = cpool.tile([P, 2 * P], mybir.dt.int32)
            iota_b = cpool.tile([P, 2 * P], mybir.dt.bfloat16)
            iota_rep = cpool.tile([P, SEL_PIECE, 2 * P], mybir.dt.bfloat16)

            # per-supergroup meta loads; first supergroup split per call so
            # the first gather starts as early as possible
            sg_w = [sum(len(sched[s][c]) for c in range(NCHUNKS)) for s in range(nsg)]
            off = 0
            for s in range(nsg):
                w = sg_w[s]
                if w == 0:
                    continue
                step = CALL_SLOTS if s == 0 else w
                for o2 in range(off, off + w, step):
                    w2 = min(step, off + w - o2)
                    nc.sync.dma_start(out=idx_t[:, o2 * 8:(o2 + w2) * 8],
                                      in_=t_idx[:, o2 * 8:(o2 + w2) * 8])
                    nc.sync.dma_start(out=trow_t[:, o2:o2 + w2],
                                      in_=t_trow[:, o2:o2 + w2])
                off += w
            nc.gpsimd.iota(iota_i[:], pattern=[[1, 2 * P]], base=0, channel_multiplier=0)
            nc.vector.tensor_copy(out=iota_b[:], in_=iota_i[:])
            for r in range(SEL_PIECE):
                nc.vector.tensor_copy(out=iota_rep[:, r, :], in_=iota_b[:])
            # -trow in f32 for the Scalar-engine tent builds
            nc.vector.tensor_scalar(out=ntrow_t[:], in0=trow_t[:], scalar1=-1.0,
                                    scalar2=None, op0=mybir.AluOpType.mult)

            # one PSUM tile = one supergroup: region(h) = cols [256h, 256h+256)
            psum_t = ppool.tile([P, 16 * P * 2], mybir.dt.float32, name="ps", tag="ps")

            slot_off = 0     # global slot offset in the stream
            g_base = 0       # global group offset
            q_load = [0] * NQ  # per-queue slot totals (greedy balance)
            piece_ctr = 0    # sel piece counter (DVE/Scalar split)
            for s in range(nsg):
                gs = SG_SIZES[s]
                # zero this supergroup's regions on the (idle) scalar engine
                for h in range(gs):
                    nc.scalar.memzero(psum_t[:, h * 2 * P:(h + 1) * 2 * P])

                for c in _chunk_order(sched, s):
                    labels = sched[s][c]
                    w = len(labels)
                    if w == 0:
                        continue
                    for g0 in range(0, w, CALL_SLOTS):
                        gw = min(CALL_SLOTS, w - g0)
                        msgs_t = mpool.tile([P, CALL_SLOTS, ELEM], mybir.dt.bfloat16,
                                            name="msgs")
                        q = min(range(NQ), key=lambda i: q_load[i])
                        q_load[q] += gw
                        nc.gpsimd.dma_gather(
                            out_ap=msgs_t[:, :gw, :],
                            in_ap=t_x[c * CHUNK:(c + 1) * CHUNK, :],
                            idxs_ap=idx_t[:, (slot_off + g0) * 8:(slot_off + g0 + gw) * 8],
                            num_idxs=gw * P,
                            num_idxs_reg=gw * P,
                            elem_size=ELEM,
                            single_packet=False,
                            queue_num=q,
                        )
                        for p0 in range(g0, g0 + gw, SEL_PIECE):
                            pw = min(SEL_PIECE, g0 + gw - p0)
                            sel_t = spool.tile([P, SEL_PIECE, 2 * P],
                                               mybir.dt.bfloat16, name="sel")
                            on_scalar = (piece_ctr % SEL_MOD) < SCALAR_SEL
                            piece_ctr += 1
                            if on_scalar:
                                for si in range(pw):
                                    sl = slot_off + p0 + si
                                    sq_t = qpool.tile([P, 2 * P], mybir.dt.bfloat16,
                                                      name="sq")
                                    i1 = nc.scalar.activation(
                                        out=sq_t[:], in_=iota_b[:],
                                        func=mybir.ActivationFunctionType.Abs,
                                        bias=ntrow_t[:, sl:sl + 1], scale=1.0,
                                    )
                                    i2 = nc.scalar.activation(
                                        out=sel_t[:, si, :], in_=sq_t[:],
                                        func=mybir.ActivationFunctionType.Relu,
                                        bias=1.0, scale=-1.0,
                                    )
                                    SEL_SCALAR_INSTS.append(i1.ins.name)
                                    SEL_SCALAR_INSTS.append(i2.ins.name)
                            else:
                                i_sel = nc.vector.tensor_tensor(
                                    out=sel_t[:, :pw, :],
                                    in0=trow_t[:, slot_off + p0:slot_off + p0 + pw]
                                    .to_broadcast([P, pw, 2 * P]),
                                    in1=iota_rep[:, :pw, :],
                                    op=mybir.AluOpType.is_equal,
                                )
                                SEL_DVE_INSTS.append(i_sel.ins.name)
                            for si in range(pw):
                                slot = p0 + si
                                h = int(labels[slot])
                                i_mm = nc.tensor.matmul(
                                    psum_t[:, h * 2 * P:(h + 1) * 2 * P],
                                    lhsT=msgs_t[:, slot - g0, :],
                                    rhs=sel_t[:, si, :],
                                    start=False,
                                    stop=False,
                                    skip_group_check=True,
                                )
                                MM_ALL.append(i_mm.ins.name)
                    slot_off += w

                # finalize: fold hi+lo partitions and spill halves -> stage.
                # Scalar engine drains PSUM->SBUF; DVE adds SBUF-SBUF.
                stage_t = stpool.tile([F, GSG * P], mybir.dt.float32, name="stage")
                for h in range(gs):
                    dst = stage_t[:, h * P:(h + 1) * P]
                    fa = fpool.tile([F, P], mybir.dt.float32, name="fa", tag="fa")
                    fb = fpool.tile([F, P], mybir.dt.float32, name="fb", tag="fb")
                    nc.scalar.copy(out=fa[:], in_=psum_t[0:F, h * 2 * P:h * 2 * P + P])
                    nc.scalar.copy(out=fb[:], in_=psum_t[F:2 * F, h * 2 * P:h * 2 * P + P])
                    nc.vector.tensor_add(out=dst, in0=fa[:], in1=fb[:])
                    if h > 0:
                        nc.scalar.copy(
                            out=fa[:], in_=psum_t[0:F, (h - 1) * 2 * P + P:h * 2 * P])
                        nc.scalar.copy(
                            out=fb[:], in_=psum_t[F:2 * F, (h - 1) * 2 * P + P:h * 2 * P])
                        nc.vector.tensor_add(out=dst, in0=dst, in1=fa[:])
                        nc.vector.tensor_add(out=dst, in0=dst, in1=fb[:])
                nc.sync.dma_start(
                    out=t_out[:, g_base * P:(g_base + gs) * P],
                    in_=stage_t[:, :gs * P],
                )
                g_base += gs

    nc.compile()
    return nc


def kernel(x, edge_idx):
    from concourse.bass_utils import run_bass_kernel_spmd

    xp, idx_dev, trow_dev, inv_cnt, sched, tot = _host_prep(x, edge_idx)
    nc = _build_program(sched, tot)
    in_maps = [
        {"xp": xp, "idx": idx_dev[k], "trow": trow_dev[k]}
        for k in range(NCORES)
    ]
    res = run_bass_kernel_spmd(nc, in_maps, list(range(NCORES)))
    # device output is [F, TPC] per core (sums); finish mean on host
    out = np.concatenate(
        [res.results[k]["out"].T for k in range(NCORES)], axis=0
    )  # [NCORES*TPC, F]
    out *= inv_cnt[:, None]
    return np.ascontiguousarray(out[:N], dtype=np.float32)
